# revision 1
# baseline (speedup 1.0000x reference)
"""Differential attention (B=2, T=2048, D=2048, H=16, HD=128) on 8 TRN2 cores.

Sharding: core c -> (batch b = c // 4, head-group g = c % 4); each core runs
batch b with 4 heads (4g..4g+3): q/k/v/out projections sliced along the head
dim, plus its batch's rows. The out-projection partial sums (over head groups)
are reduced on the host.

Per-core kernel (one SPMD Bass program):
  projections: QT/KT (head-dim on partitions) and V (seq on partitions) via
      matmuls contracting D with x^T chunks (x^T supplied by host). t-blocks
      0-1 are projected up front; t-blocks 2-3 are interleaved into the
      attention stream (they are only needed by the later q-superblocks), so
      the PE stays busy while attention waits on softmax.
  attention: causal diff-attention per (head, 512-row q-block): scores via
      row-packed K=64 matmul pairs, exp on ScalarE with fused row-sum
      accumulation (diagonal blocks masked 0/1 in bf16 and re-summed on
      VectorE), combine (attn1 - sigmoid(lambda)*attn2) on VectorE,
      PE-transpose of the combined weights, attn @ V, then the out-projection
      contracting the local heads.

All matmuls run in bf16 (inputs rounded on host / at PSUM evacuation);
accumulation is fp32 in PSUM, softmax statistics are fp32.
"""

from contextlib import ExitStack

import ml_dtypes
import numpy as np

B, T, D = 2, 2048, 2048
H, HD = 16, 128
HHD = HD // 2
HL = 4  # heads per core
NCORES = 8
SCALE = 1.0 / float(np.sqrt(np.float32(HHD)))

TB = 512  # t-superblock (q-block rows, AV free dim)
NTB = T // TB  # 4
DC = 128  # contraction chunk (partition dim)
NDC = D // DC  # 16
NQT = TB // 128  # q-tiles (128 rows) per superblock
NST = T // 128  # s-subchunks over full T
CH = 512  # softmax chunk width (1 PSUM bank)

_CACHE = {}


def _build():
    import concourse.mybir as mybir
    from concourse.bacc import Bacc
    from concourse.tile import TileContext

    f32 = mybir.dt.float32
    bf16 = mybir.dt.bfloat16
    Alu = mybir.AluOpType
    Act = mybir.ActivationFunctionType
    X = mybir.AxisListType.X

    nc = Bacc("TRN2", num_devices=NCORES)
    xt = nc.declare_dram_parameter("xt", [D, T], bf16, isOutput=False)
    wq = nc.declare_dram_parameter("wq", [D, HL * HD], bf16, isOutput=False)
    wk = nc.declare_dram_parameter("wk", [D, HL * HD], bf16, isOutput=False)
    wv = nc.declare_dram_parameter("wv", [D, HL * HD], bf16, isOutput=False)
    won = nc.declare_dram_parameter("won", [HL * HD, D], bf16, isOutput=False)
    lam = nc.declare_dram_parameter("lam", [128, HL], f32, isOutput=False)
    msk = nc.declare_dram_parameter("msk", [128, 128], f32, isOutput=False)
    idn = nc.declare_dram_parameter("idn", [128, 128], bf16, isOutput=False)
    out = nc.declare_dram_parameter("out", [T, D], f32, isOutput=True)

    with TileContext(nc) as tc, ExitStack() as top:
        const = top.enter_context(tc.tile_pool(name="const", bufs=1))
        lam_sb = const.tile([128, HL], f32, tag="lam", name="lam")
        msk_sb = const.tile([128, 128], f32, tag="msk", name="msk")
        idn_sb = const.tile([128, 128], bf16, tag="idn", name="idn")
        nc.sync.dma_start(out=lam_sb[:], in_=lam[:])
        nc.sync.dma_start(out=msk_sb[:], in_=msk[:])
        nc.sync.dma_start(out=idn_sb[:], in_=idn[:])

        resid = top.enter_context(tc.tile_pool(name="resid", bufs=1))
        qt_sb = [resid.tile([128, T], bf16, tag=f"qt{h}", name=f"qt{h}") for h in range(HL)]
        kt_sb = [resid.tile([128, T], bf16, tag=f"kt{h}", name=f"kt{h}") for h in range(HL)]
        v_sb = [resid.tile([128, HL * HD], bf16, tag=f"v{s}", name=f"v{s}") for s in range(NST)]

        # weights + x^T pools stay open for the whole kernel (t-blocks 2-3
        # are projected in the middle of the attention stream)
        wpool = top.enter_context(tc.tile_pool(name="wpool", bufs=1))
        wq_sb = [wpool.tile([128, HL * HD], bf16, tag=f"wq{d}", name=f"wq{d}") for d in range(NDC)]
        wk_sb = [wpool.tile([128, HL * HD], bf16, tag=f"wk{d}", name=f"wk{d}") for d in range(NDC)]
        wv_sb = [wpool.tile([128, HL * HD], bf16, tag=f"wv{d}", name=f"wv{d}") for d in range(NDC)]
        for d in range(NDC):
            dsl = slice(d * DC, (d + 1) * DC)
            nc.sync.dma_start(out=wq_sb[d][:], in_=wq[dsl, :])
            nc.sync.dma_start(out=wk_sb[d][:], in_=wk[dsl, :])
            nc.sync.dma_start(out=wv_sb[d][:], in_=wv[dsl, :])
        xpool = top.enter_context(tc.tile_pool(name="xpool", bufs=20))

        _xts = {}

        def emit_proj_part(tb, part, psp, ptag):
            """One third of a 512-row t-block's projections.
            part 0: x-loads + Q, part 1: K, part 2: V (frees the x tiles)."""
            tsl = slice(tb * TB, (tb + 1) * TB)
            if part == 0:
                xts = []
                for d in range(NDC):
                    t = xpool.tile([128, TB], bf16, tag="xt", name="xt")
                    nc.sync.dma_start(
                        out=t[:], in_=xt[d * DC : (d + 1) * DC, tsl]
                    )
                    xts.append(t)
                _xts[tb] = xts
            xts = _xts[tb]
            if part in (0, 1):
                w_sb, dst = ((wq_sb, qt_sb), (wk_sb, kt_sb))[part]
                for h in range(HL):
                    ps = psp.tile([128, TB], f32, tag=ptag, name=ptag)
                    hsl = slice(h * HD, (h + 1) * HD)
                    for d in range(NDC):
                        nc.tensor.matmul(
                            ps[:], lhsT=w_sb[d][:, hsl], rhs=xts[d][:],
                            start=(d == 0), stop=(d == NDC - 1),
                        )
                    nc.any.tensor_copy(dst[h][:, tsl], ps[:])
            else:
                for tt in range(NQT):
                    ps = psp.tile([128, HL * HD], f32, tag=ptag, name=ptag)
                    ttsl = slice(tt * 128, (tt + 1) * 128)
                    for d in range(NDC):
                        nc.tensor.matmul(
                            ps[:], lhsT=xts[d][:, ttsl], rhs=wv_sb[d][:],
                            start=(d == 0), stop=(d == NDC - 1),
                        )
                    nc.any.tensor_copy(v_sb[tb * NQT + tt][:], ps[:])
                del _xts[tb]

        def emit_proj_tb(tb, psp, ptag):
            for part in range(3):
                emit_proj_part(tb, part, psp, ptag)

        # ---- up-front projections: t-blocks 0-1 (full 8-bank pipeline) ----
        with ExitStack() as ph1:
            pps = ph1.enter_context(tc.tile_pool(name="pps", bufs=8, space="PSUM"))
            emit_proj_tb(0, pps, "proj")
            emit_proj_tb(1, pps, "proj")

        # ---------------- attention + out-projection ----------------
        with ExitStack() as ph2:
            scps = ph2.enter_context(tc.tile_pool(name="scps", bufs=2, space="PSUM"))
            atps = ph2.enter_context(tc.tile_pool(name="atps", bufs=1, space="PSUM"))
            accps = ph2.enter_context(tc.tile_pool(name="accps", bufs=2, space="PSUM"))
            prjps = ph2.enter_context(tc.tile_pool(name="prjps", bufs=1, space="PSUM"))
            epool = ph2.enter_context(tc.tile_pool(name="epool", bufs=10))
            tmpp = ph2.enter_context(tc.tile_pool(name="tmpp", bufs=5))
            dpool = ph2.enter_context(tc.tile_pool(name="dpool", bufs=18))
            apool = ph2.enter_context(tc.tile_pool(name="apool", bufs=6))
            opool = ph2.enter_context(tc.tile_pool(name="opool", bufs=4))
            spool = ph2.enter_context(tc.tile_pool(name="spool", bufs=3))
            otp = ph2.enter_context(tc.tile_pool(name="otp", bufs=1))
            ot_sb = [otp.tile([128, T], bf16, tag=f"ot{h}", name=f"ot{h}") for h in range(HL)]
            wo_sb = [otp.tile([128, D], bf16, tag=f"wo{h}", name=f"wo{h}") for h in range(HL)]
            for h in range(HL):
                nc.sync.dma_start(out=wo_sb[h][:], in_=won[h * 128 : (h + 1) * 128, :])

            def emit_attn_qsb(qsb, fillers=()):
                s_end = (qsb + 1) * TB
                nsc = s_end // 128  # s-subchunks for transposes/AV
                for h in range(HL):
                    if h < len(fillers) and fillers[h] is not None:
                        fillers[h]()
                    q1 = qt_sb[h][0:64, :]
                    q2 = qt_sb[h][64:128, :]
                    k1 = kt_sb[h][0:64, :]
                    k2 = kt_sb[h][64:128, :]
                    diffs = []
                    for qt in range(NQT):
                        tq0 = qsb * TB + qt * 128
                        qsl = slice(tq0, tq0 + 128)
                        S = tq0 + 128  # causal row limit for this q-tile
                        nch = (S + CH - 1) // CH
                        l1p = spool.tile([128, 4], f32, tag="l1p", name="l1p")
                        l2p = spool.tile([128, 4], f32, tag="l2p", name="l2p")
                        chunks = [None] * nch
                        # diagonal (DVE-heavy) chunk first: its mask+row-sum
                        # overlaps the other chunks' exps on ScalarE
                        for c in [nch - 1] + list(range(nch - 1)):
                            w = min(CH, S - c * CH)
                            ps1 = scps.tile([128, CH], f32, tag="ps1", name="ps1")
                            ps2 = scps.tile([128, CH], f32, tag="ps2", name="ps2")
                            for j in range(0, w, 512):
                                jw = min(512, w - j)
                                nc.tensor.matmul(
                                    ps1[:, j : j + jw], lhsT=q1[:, qsl],
                                    rhs=k1[:, c * CH + j : c * CH + j + jw],
                                    start=True, stop=True,
                                )
                                nc.tensor.matmul(
                                    ps2[:, j : j + jw], lhsT=q2[:, qsl],
                                    rhs=k2[:, c * CH + j : c * CH + j + jw],
                                    start=True, stop=True,
                                )
                            if c == nch - 1:
                                # additive causal mask (-1e30) on the diagonal
                                # 128 columns, in PSUM before exp, so every
                                # exp can carry the fused row-sum accumulator
                                dw = (S - 128) - c * CH
                                nc.vector.tensor_add(
                                    ps1[:, dw : dw + 128], ps1[:, dw : dw + 128],
                                    msk_sb[:],
                                )
                                nc.vector.tensor_add(
                                    ps2[:, dw : dw + 128], ps2[:, dw : dw + 128],
                                    msk_sb[:],
                                )
                            e1 = epool.tile([128, CH], bf16, tag="e1", name="e1")
                            e2 = epool.tile([128, CH], bf16, tag="e2", name="e2")
                            nc.scalar.activation(
                                e1[:, :w], ps1[:, :w], Act.Exp, scale=SCALE,
                                accum_out=l1p[:, c : c + 1],
                            )
                            nc.scalar.activation(
                                e2[:, :w], ps2[:, :w], Act.Exp, scale=SCALE,
                                accum_out=l2p[:, c : c + 1],
                            )
                            chunks[c] = (e1, e2, w)

                        # per-q-tile softmax scalars, then combine
                        r1 = spool.tile([128, 1], f32, tag="r1", name="r1")
                        r2t = spool.tile([128, 1], f32, tag="r2t", name="r2t")
                        r2 = spool.tile([128, 1], f32, tag="r2", name="r2")
                        if nch > 1:
                            ls1 = spool.tile([128, 1], f32, tag="ls1", name="ls1")
                            ls2 = spool.tile([128, 1], f32, tag="ls2", name="ls2")
                            nc.vector.reduce_sum(ls1[:], l1p[:, :nch], axis=X)
                            nc.vector.reduce_sum(ls2[:], l2p[:, :nch], axis=X)
                            src1, src2 = ls1[:], ls2[:]
                        else:
                            src1, src2 = l1p[:, 0:1], l2p[:, 0:1]
                        nc.vector.reciprocal(r1[:], src1)
                        nc.vector.reciprocal(r2t[:], src2)
                        nc.vector.tensor_scalar(
                            r2[:], r2t[:], lam_sb[:, h : h + 1], None, Alu.mult
                        )
                        dchunks = []
                        for c in range(nch):
                            e1, e2, w = chunks[c]
                            tmp = tmpp.tile([128, CH], bf16, tag="tmp", name="tmp")
                            nc.vector.tensor_scalar(
                                tmp[:, :w], e2[:, :w], r2[:], None, Alu.mult
                            )
                            dn = dpool.tile([128, CH], bf16, tag="dn", name="dn")
                            nc.vector.scalar_tensor_tensor(
                                dn[:, :w], e1[:, :w], r1[:], tmp[:, :w],
                                Alu.mult, Alu.subtract,
                            )
                            dchunks.append((dn, w))
                        diffs.append(dchunks)

                    # transposes + attn @ V for this (h, qsb)
                    av = accps.tile([128, TB], f32, tag="acc", name="acc")
                    for k in range(nsc):
                        j0 = 0 if k < qsb * NQT else (k - qsb * NQT)
                        aT = atps.tile([128, TB], bf16, tag="aT", name="aT")
                        for qt in range(j0, NQT):
                            c, off = divmod(k * 128, CH)
                            dn, _w = diffs[qt][c]
                            nc.tensor.transpose(
                                aT[:, qt * 128 : (qt + 1) * 128],
                                dn[:, off : off + 128],
                                idn_sb[:],
                            )
                        aTs = apool.tile([128, TB], bf16, tag="aTs", name="aTs")
                        nc.vector.tensor_copy(aTs[:, j0 * 128 :], aT[:, j0 * 128 :])
                        nc.tensor.matmul(
                            av[:, j0 * 128 :],
                            lhsT=v_sb[k][:, h * HD : (h + 1) * HD],
                            rhs=aTs[:, j0 * 128 :],
                            start=(k == 0),
                            stop=(k == nsc - 1),
                        )
                    nc.vector.tensor_copy(ot_sb[h][:, qsb * TB : (qsb + 1) * TB], av[:])

            def emit_outproj_qsb(qsb):
                # out-projection for a q-superblock (after all 4 heads)
                for tt in range(NQT):
                    t0 = qsb * TB + tt * 128
                    for dch in range(4):
                        dsl = slice(dch * 512, (dch + 1) * 512)
                        po = accps.tile([128, 512], f32, tag="acc", name="acc")
                        for h in range(HL):
                            nc.tensor.matmul(
                                po[:],
                                lhsT=ot_sb[h][:, t0 : t0 + 128],
                                rhs=wo_sb[h][:, dsl],
                                start=(h == 0),
                                stop=(h == HL - 1),
                            )
                        oev = opool.tile([128, 512], f32, tag="oev", name="oev")
                        nc.any.tensor_copy(oev[:], po[:])
                        nc.sync.dma_start(out=out[t0 : t0 + 128, dsl], in_=oev[:])

            # interleave independent PE work (remaining projections, earlier
            # q-superblocks' out-projections) between attention heads so the
            # PE never drains while softmax chains run on ScalarE/VectorE
            emit_attn_qsb(0)
            emit_attn_qsb(
                1,
                fillers=[
                    lambda: emit_proj_part(2, 0, prjps, "projb"),
                    lambda: emit_proj_part(2, 1, prjps, "projb"),
                    lambda: emit_proj_part(2, 2, prjps, "projb"),
                    lambda: emit_outproj_qsb(0),
                ],
            )
            emit_attn_qsb(
                2,
                fillers=[
                    lambda: emit_proj_part(3, 0, prjps, "projb"),
                    lambda: emit_proj_part(3, 1, prjps, "projb"),
                    lambda: emit_proj_part(3, 2, prjps, "projb"),
                    lambda: emit_outproj_qsb(1),
                ],
            )
            emit_attn_qsb(3, fillers=[None, lambda: emit_outproj_qsb(2), None, None])
            emit_outproj_qsb(3)

    nc.finalize()
    return nc


def _get_nc():
    if "nc" not in _CACHE:
        _CACHE["nc"] = _build()
    return _CACHE["nc"]


def kernel(x, Wq, Wk, Wv, Wo, lambda_init):
    from concourse.bass_utils import run_bass_kernel_spmd

    bf16 = ml_dtypes.bfloat16
    x = np.asarray(x, dtype=np.float32)
    Wq = np.asarray(Wq, dtype=np.float32)
    Wk = np.asarray(Wk, dtype=np.float32)
    Wv = np.asarray(Wv, dtype=np.float32)
    Wo = np.asarray(Wo, dtype=np.float32)
    lam_full = 1.0 / (1.0 + np.exp(-np.asarray(lambda_init, dtype=np.float32)))

    msk = np.triu(np.full((128, 128), -1e30, np.float32), k=1)  # additive causal
    idn = np.eye(128, dtype=bf16)

    xt_b = [np.ascontiguousarray(x[b].T).astype(bf16) for b in range(B)]
    in_maps = []
    for c in range(NCORES):
        b, g = divmod(c, NCORES // B)  # b = c // 4, g = c % 4
        cols = slice(g * HL * HD, (g + 1) * HL * HD)
        in_maps.append(
            {
                "xt": xt_b[b],
                "wq": np.ascontiguousarray(Wq[:, cols]).astype(bf16),
                "wk": np.ascontiguousarray(Wk[:, cols]).astype(bf16),
                "wv": np.ascontiguousarray(Wv[:, cols]).astype(bf16),
                "won": np.ascontiguousarray(Wo[cols, :]).astype(bf16),
                "lam": np.tile(lam_full[g * HL : (g + 1) * HL], (128, 1)).astype(
                    np.float32
                ),
                "msk": msk,
                "idn": idn,
            }
        )

    nc = _get_nc()
    res = run_bass_kernel_spmd(nc, in_maps, core_ids=list(range(NCORES)))
    _CACHE["last_results"] = res  # exec_time_ns etc. when tracing is enabled

    full = np.zeros((B, T, D), np.float32)
    for c in range(NCORES):
        b = c // (NCORES // B)
        full[b] += res.results[c]["out"]
    return full

